# revision 44
# baseline (speedup 1.0000x reference)
"""ODE-LSTM cell (LSTMCell + RK4 ODE trajectory + per-row time gather) on 8 trn2 cores.

Strategy
--------
Data-parallel over batch: each of the 8 cores integrates 32 of the 256 batch
rows through the full 255-step RK4 scan with replicated weights (the ODE is
autonomous and rows are independent, so there is zero inter-core traffic).

On-chip layout is "transposed": H lives on (k-tile, partition) and batch on the
free dim, i.e. tensor[p, k*32 + b] = x[b, k*128 + p].  Both MLP GEMMs are then
weight-stationary (lhsT = W.T tile [K=128, M=128], rhs = state [K=128, N=32])
and produce outputs in the same layout, so the whole RK4 loop runs without a
single transpose.

The time grid (argsort(ts) + linspace + diff) is data-dependent scheduling, so
it is computed on the host and baked in: dts go in as a small SBUF table
indexed by the hardware loop variable; the output gather becomes a static
permutation.  Batch rows are permuted so that core c, local slot j holds the
row whose snapshot time is t = 32c + j.  Every core then runs the *same*
program: at step t it copies local column (t mod 32) into a snapshot
accumulator at column t.  The valid recording for core c, slot j is the one at
t = 32c + j; the host picks those out and inverse-permutes.
"""

import numpy as np

import concourse.bacc as bacc
import concourse.bass as bass
import concourse.mybir as mybir
import concourse.tile as tile
from concourse.bass import ds
from concourse.bass import MemorySpace
from concourse.bass_utils import run_bass_kernel_spmd

AF = mybir.ActivationFunctionType
ALU = mybir.AluOpType
FP32 = mybir.dt.float32
FP16 = mybir.dt.float16

# dtype of the ODE-loop matmul operands (weights + moving activations).
# PSUM accumulation and the RK4 state stay fp32 either way.  fp16 weights
# enable Fast Weight Load (2x LDWEIGHTS) and 1 cycle/row matmuls vs 4 for
# fp32.
import os as _os
MM_FP16 = _os.environ.get("MM_FP16", "1") == "1"
# emit the inter-eval combine ops in 4 per-m-tile slices so the next eval's
# K-accumulation can start as soon as its slice is ready
SLICED = _os.environ.get("SLICED", "0") == "1"
# For_i back-edge tuning
STAG = _os.environ.get("STAG", "0") == "1"
HINT = _os.environ.get("HINT", "0") == "1"
UNROLL = int(_os.environ.get("UNROLL", "1"))  # steps per For_i iteration
# FOLD: skip materializing k_i; feed z-PSUM straight into fused DVE combines
# with the b2 bias folded algebraically (hn = s + dtS*(z1+2z2+2z3+z4) + dt*b2)
FOLD = _os.environ.get("FOLD", "0") == "1"
# PEONLY: timing probe — emit only the matmul stream of each step
PEONLY = _os.environ.get("PEONLY", "0") == "1"
# KOUTER: emit GEMM2 with k as the outer loop so the tail tanh's latency is
# hidden behind the k<3 matmul groups
KOUTER = _os.environ.get("KOUTER", "0") == "1"
# DVEID: do the k_i = z + b2 bias-add on DVE (static per-partition scalar)
# instead of a ScalarE Identity activation
DVEID = _os.environ.get("DVEID", "0") == "1"
# SNAPG: issue the per-step snapshot copy on GpSimd instead of DVE
SNAPG = _os.environ.get("SNAPG", "0") == "1"

B, IN, H = 256, 128, 512
NCORES = 8
BC = B // NCORES          # 32 batch rows per core
KT = H // 128             # 4 H-tiles
NSTEP = B - 1             # 255 RK4 steps
NSNAP = B                 # 256 snapshot times (t=0 is h0)

_CACHE = {}
_RUN_KWARGS = {}      # test harness may set {"trace": True}
_LAST_RESULT = []     # test harness reads BassKernelResults from here


def _build_program(n_outer=NSNAP // BC):
    nc = bacc.Bacc("TRN2", target_bir_lowering=False, debug=False,
                   num_devices=NCORES)

    mmdt = FP16 if MM_FP16 else FP32

    # ---- DRAM I/O ----
    din = {}
    def dram_in(name, shape, dt=FP32):
        din[name] = nc.dram_tensor(name, shape, dt, kind="ExternalInput").ap()
    dram_in("xinT", [128, BC])            # [p, b] = inputs[row_b, p]
    dram_in("hT",   [128, KT * BC])       # [p, k*32+b] = h[row_b, k*128+p]
    dram_in("cT",   [128, KT * BC])
    dram_in("wih",  [128, 16 * 128])      # blk m: W_ih[m*128:(m+1)*128, :].T
    dram_in("whh",  [128, 64 * 128])      # blk m*4+k: W_hh[m128, k128].T
    dram_in("w1",   [128, 16 * 128], mmdt)  # blk m*4+k: W1[m128, k128].T
    dram_in("w2",   [128, 16 * 128], mmdt)
    dram_in("bg",   [128, 16])            # [p, m] = (b_ih+b_hh)[m*128+p]
    dram_in("b1",   [128, KT])
    dram_in("b2",   [128, KT])
    dram_in("b2bc", [128, KT * BC])       # b2 broadcast along batch
    dram_in("dt3",  [128, 3 * NSNAP])     # cols: [dt/2 | dt | dt/6], padded

    snap_d = nc.dram_tensor("snap", [128, KT * NSNAP], FP32,
                            kind="ExternalOutput").ap()
    newc_d = nc.dram_tensor("newc", [128, KT * BC], FP32,
                            kind="ExternalOutput").ap()

    with tile.TileContext(nc) as tc:
        with (
            tc.tile_pool(name="consts", bufs=1) as consts,
            tc.tile_pool(name="state", bufs=1) as statep,
            tc.tile_pool(name="work", bufs=2) as work,
            tc.tile_pool(name="kbuf", bufs=1) as kbuf,
            tc.tile_pool(name="psum", bufs=6 if FOLD else 8,
                         space=MemorySpace.PSUM) as psum,
            tc.tile_pool(name="psumz", bufs=2, space=MemorySpace.PSUM) as psumz,
        ):
            # ---- load constants ----
            sb = {}
            for name in ("xinT", "hT", "cT", "wih", "whh", "w1", "w2",
                         "bg", "b1", "b2", "b2bc", "dt3"):
                t = consts.tile(list(din[name].shape), din[name].dtype, tag=name)
                nc.sync.dma_start(t[:], din[name][:])
                sb[name] = t

            s = statep.tile([128, KT * BC], FP32, tag="s")          # ODE state
            snap = statep.tile([128, KT * NSNAP], FP32, tag="snap")  # recorder
            nc.gpsimd.memset(snap[:], 0.0)

            # ---- LSTM cell ----
            # gates.T[m-tile] = W_ih[m]@x.T + W_hh[m,:]@h.T  (bias in ACT)
            gact = {}  # m -> sbuf gate tile [128, BC]
            for m in range(16):
                g = psum.tile([128, BC], FP32, tag="psum")
                nc.tensor.matmul(g[:], sb["wih"][:, m * 128:(m + 1) * 128],
                                 sb["xinT"][:], start=True, stop=False)
                for k in range(KT):
                    nc.tensor.matmul(
                        g[:], sb["whh"][:, (m * KT + k) * 128:(m * KT + k + 1) * 128],
                        sb["hT"][:, k * BC:(k + 1) * BC],
                        start=False, stop=(k == KT - 1))
                func = AF.Tanh if 8 <= m < 12 else AF.Sigmoid
                ga = work.tile([128, BC], FP32, tag=f"gate{m}")
                nc.scalar.activation(ga[:], g[:], func, bias=sb["bg"][:, m:m + 1])
                gact[m] = ga

            ncT = statep.tile([128, KT * BC], FP32, tag="ncT")
            for k in range(KT):
                co = slice(k * BC, (k + 1) * BC)
                v1 = work.tile([128, BC], FP32, tag="v1")
                nc.vector.tensor_mul(v1[:], gact[4 + k][:], sb["cT"][:, co])
                v2 = work.tile([128, BC], FP32, tag="v2")
                nc.vector.tensor_mul(v2[:], gact[k][:], gact[8 + k][:])
                nc.vector.tensor_add(ncT[:, co], v1[:], v2[:])
                tnc = work.tile([128, BC], FP32, tag="tnc")
                nc.scalar.activation(tnc[:], ncT[:, co], AF.Tanh)
                nc.vector.tensor_mul(s[:, co], gact[12 + k][:], tnc[:])
            nc.sync.dma_start(newc_d[:], ncT[:])

            # fp16 shadow of the state for matmul rhs
            if MM_FP16:
                s16 = statep.tile([128, KT * BC], FP16, tag="s16")
                nc.vector.tensor_copy(s16[:], s[:])
            else:
                s16 = s

            # ---- ODE vector field: kout = W2.T@tanh(W1.T@x + b1) + b2 ----
            def gemm1_tanh(x_ap):
                u = work.tile([128, KT * BC], mmdt, tag="u")
                for m in range(KT):
                    y = psum.tile([128, BC], FP32, tag="psum")
                    for k in range(KT):
                        nc.tensor.matmul(
                            y[:], sb["w1"][:, (m * KT + k) * 128:(m * KT + k + 1) * 128],
                            x_ap[:, k * BC:(k + 1) * BC],
                            start=(k == 0), stop=(k == KT - 1))
                    nc.scalar.activation(u[:, m * BC:(m + 1) * BC], y[:],
                                         AF.Tanh, bias=sb["b1"][:, m:m + 1])
                return u

            def f_eval(x_ap, kout):
                u = gemm1_tanh(x_ap)
                if KOUTER:
                    zs = []
                    for _zi in range(KT):
                        zt = psum.tile([128, BC], FP32, tag="psum", name=f"z{_zi}")
                        zs.append(zt)
                    for k in range(KT):
                        for m in range(KT):
                            nc.tensor.matmul(
                                zs[m][:],
                                sb["w2"][:, (m * KT + k) * 128:(m * KT + k + 1) * 128],
                                u[:, k * BC:(k + 1) * BC],
                                start=(k == 0), stop=(k == KT - 1))
                    for m in range(KT):
                        nc.scalar.activation(kout[:, m * BC:(m + 1) * BC],
                                             zs[m][:], AF.Identity,
                                             bias=sb["b2"][:, m:m + 1])
                else:
                    for m in range(KT):
                        z = psum.tile([128, BC], FP32, tag="psum")
                        for k in range(KT):
                            nc.tensor.matmul(
                                z[:], sb["w2"][:, (m * KT + k) * 128:(m * KT + k + 1) * 128],
                                u[:, k * BC:(k + 1) * BC],
                                start=(k == 0), stop=(k == KT - 1))
                        nc.scalar.activation(kout[:, m * BC:(m + 1) * BC], z[:],
                                             AF.Identity, bias=sb["b2"][:, m:m + 1])

            def f_eval_fold(x_ap):
                # z left in PSUM as one [128, KT*BC] tile (single bank)
                u = gemm1_tanh(x_ap)
                z = psumz.tile([128, KT * BC], FP32, tag="zbig")
                order = ([(k, m) for k in range(KT) for m in range(KT)]
                         if KOUTER else
                         [(k, m) for m in range(KT) for k in range(KT)])
                for k, m in order:
                    nc.tensor.matmul(
                        z[:, m * BC:(m + 1) * BC],
                        sb["w2"][:, (m * KT + k) * 128:(m * KT + k + 1) * 128],
                        u[:, k * BC:(k + 1) * BC],
                        start=(k == 0), stop=(k == KT - 1))
                return z

            k1 = kbuf.tile([128, KT * BC], FP32, tag="k1")
            k2 = kbuf.tile([128, KT * BC], FP32, tag="k2")
            k3 = kbuf.tile([128, KT * BC], FP32, tag="k3")
            k4 = kbuf.tile([128, KT * BC], FP32, tag="k4")

            s3 = s[:].rearrange("p (k b) -> p k b", k=KT)
            snap3 = snap[:].rearrange("p (k t) -> p k t", k=KT)

            def snapshot(jv, tv):
                # record local column (t mod 32) at snapshot slot t
                eng = nc.gpsimd if SNAPG else nc.vector
                eng.tensor_copy(snap3[:, :, ds(tv, 1)], s3[:, :, ds(jv, 1)])

            def stt(out, in0, scal, in1):
                # out = in0 * scal + in1
                nc.vector.scalar_tensor_tensor(out, in0, scal, in1,
                                               ALU.mult, ALU.add)

            slices = ([slice(m * BC, (m + 1) * BC) for m in range(KT)]
                      if SLICED else [slice(0, KT * BC)])

            acc = statep.tile([128, KT * BC], FP32, tag="acc")

            def step_body_fold(jv, tv):
                snapshot(jv, tv)
                dtH = sb["dt3"][:, ds(tv, 1)]
                dtF = sb["dt3"][:, ds(tv + NSNAP, 1)]
                dtS = sb["dt3"][:, ds(tv + 2 * NSNAP, 1)]
                # sb2H = s + dtH*b2 ; sb2F = s + dtF*b2 (also the final base)
                sb2H = work.tile([128, KT * BC], FP32, tag="sb2H")
                sb2F = work.tile([128, KT * BC], FP32, tag="sb2F")
                stt(sb2H[:], sb["b2bc"][:], dtH, s[:])
                stt(sb2F[:], sb["b2bc"][:], dtF, s[:])
                z1 = f_eval_fold(s16)
                x2 = work.tile([128, KT * BC], mmdt, tag="x")
                stt(x2[:], z1[:], dtH, sb2H[:])
                nc.vector.tensor_copy(acc[:], z1[:])
                z2 = f_eval_fold(x2)
                x3 = work.tile([128, KT * BC], mmdt, tag="x")
                stt(x3[:], z2[:], dtH, sb2H[:])
                nc.vector.scalar_tensor_tensor(acc[:], z2[:], 2.0, acc[:],
                                               ALU.mult, ALU.add)
                z3 = f_eval_fold(x3)
                x4 = work.tile([128, KT * BC], mmdt, tag="x")
                stt(x4[:], z3[:], dtF, sb2F[:])
                nc.vector.scalar_tensor_tensor(acc[:], z3[:], 2.0, acc[:],
                                               ALU.mult, ALU.add)
                z4 = f_eval_fold(x4)
                nc.vector.tensor_add(acc[:], acc[:], z4[:])
                # hn = s + dtS*acc + dt*b2 = sb2F + dtS*acc
                if MM_FP16:
                    stt(s16[:], acc[:], dtS, sb2F[:])
                stt(s[:], acc[:], dtS, sb2F[:])

            def step_body_peonly(jv, tv):
                # pure matmul stream: 128 MMs as in a real step, no ACT/DVE
                for g in range(8):
                    wsb = sb["w1"] if g % 2 == 0 else sb["w2"]
                    for m in range(KT):
                        y = psum.tile([128, BC], FP32, tag="psum")
                        for k in range(KT):
                            nc.tensor.matmul(
                                y[:], wsb[:, (m * KT + k) * 128:(m * KT + k + 1) * 128],
                                s16[:, k * BC:(k + 1) * BC],
                                start=(k == 0), stop=(k == KT - 1))

            def step_body(jv, tv):
                snapshot(jv, tv)
                dtH = sb["dt3"][:, ds(tv, 1)]
                dtF = sb["dt3"][:, ds(tv + NSNAP, 1)]
                dtS = sb["dt3"][:, ds(tv + 2 * NSNAP, 1)]
                f_eval(s16, k1)
                x2 = work.tile([128, KT * BC], mmdt, tag="x")
                for sl in slices:
                    stt(x2[:, sl], k1[:, sl], dtH, s[:, sl])
                f_eval(x2, k2)
                x3 = work.tile([128, KT * BC], mmdt, tag="x")
                for sl in slices:
                    stt(x3[:, sl], k2[:, sl], dtH, s[:, sl])
                f_eval(x3, k3)
                x4 = work.tile([128, KT * BC], mmdt, tag="x")
                for sl in slices:
                    stt(x4[:, sl], k3[:, sl], dtF, s[:, sl])
                f_eval(x4, k4)
                c1 = work.tile([128, KT * BC], FP32, tag="c")
                c2 = work.tile([128, KT * BC], FP32, tag="c")
                c3 = work.tile([128, KT * BC], FP32, tag="c")
                for sl in slices:
                    nc.vector.tensor_add(c1[:, sl], k2[:, sl], k3[:, sl])
                    nc.vector.scalar_tensor_tensor(c2[:, sl], c1[:, sl], 2.0,
                                                   k1[:, sl], ALU.mult, ALU.add)
                    nc.vector.tensor_add(c3[:, sl], c2[:, sl], k4[:, sl])
                    stt(s[:, sl], c3[:, sl], dtS, s[:, sl])
                    if MM_FP16:
                        nc.vector.tensor_copy(s16[:, sl], s[:, sl])

            if PEONLY:
                step_body = step_body_peonly
            elif FOLD:
                step_body = step_body_fold

            # steps t = o*32 + j; last outer block stops at t=254
            static_steps = int(_os.environ.get("STATIC_STEPS", "0"))
            repeat = int(_os.environ.get("REPEAT", "0"))  # timing-only knob
            if static_steps:
                for t in range(static_steps):
                    step_body(t % BC, t)
            else:
                loop_kw = dict(staggered_reset=STAG)
                if HINT:
                    loop_kw["hint_engines"] = (mybir.EngineType.PE,
                                               mybir.EngineType.Activation,
                                               mybir.EngineType.DVE)

                def ode_loops():
                    for o in range(n_outer):
                        # o >= 8 only occurs in timing builds; wrap indices so
                        # table lookups stay in bounds (results then unused)
                        oo = o % (NSNAP // BC)
                        n_inner = BC if o < NSNAP // BC - 1 else BC - 1
                        n_loop = (n_inner // UNROLL) * UNROLL
                        with tc.For_i(0, n_loop, UNROLL, **loop_kw) as j:
                            for u in range(UNROLL):
                                step_body(j + u, j + u + oo * BC)
                        for t in range(n_loop, n_inner):  # remainder, static
                            step_body(t, t + oo * BC)

                if repeat:
                    with tc.For_i(0, repeat, 1):
                        ode_loops()
                else:
                    ode_loops()

            # final snapshot at t = 255 (static)
            nc.vector.tensor_copy(snap3[:, :, ds(NSNAP - 1, 1)],
                                  s3[:, :, ds(BC - 1, 1)])

            nc.sync.dma_start(snap_d[:], snap[:])

    nc.compile()
    return nc


def _pack_weight_blocks(W, n_m, n_k):
    # out[p, (m*n_k+k)*128 + q] = W[m*128+q, k*128+p]
    blocks = []
    for m in range(n_m):
        for k in range(n_k):
            blocks.append(W[m * 128:(m + 1) * 128, k * 128:(k + 1) * 128].T)
    return np.ascontiguousarray(np.concatenate(blocks, axis=1), dtype=np.float32)


def _to_hb(x):
    # [32, 512] row-shard -> [128, k*32+b] transposed layout
    return np.ascontiguousarray(
        x.T.reshape(KT, 128, BC).transpose(1, 0, 2).reshape(128, KT * BC),
        dtype=np.float32)


def _prep(inputs, h, c, ts, W_ih, W_hh, b_ih, b_hh, W1, b1, W2, b2):
    inputs, h, c, ts = (np.asarray(a, np.float32) for a in (inputs, h, c, ts))
    W_ih, W_hh, W1, W2 = (np.asarray(a, np.float32) for a in (W_ih, W_hh, W1, W2))
    b_ih, b_hh, b1, b2 = (np.asarray(a, np.float32) for a in (b_ih, b_hh, b1, b2))

    # ---- host: time grid + permutation ----
    indices = np.argsort(ts, kind="stable")
    s_sort = ts[indices] + np.linspace(0.0, 1e-4, B).astype(np.float32)
    s_sort = s_sort.astype(np.float32)
    dts = (s_sort[1:] - s_sort[:-1]).astype(np.float32)   # [255]
    perm = np.argsort(indices, kind="stable")  # perm[t] = row snapshotted at t

    dt3 = np.zeros((3 * NSNAP,), np.float32)
    dt3[0:NSTEP] = np.float32(0.5) * dts
    dt3[NSNAP:NSNAP + NSTEP] = dts
    dt3[2 * NSNAP:2 * NSNAP + NSTEP] = dts / np.float32(6.0)

    mmdt = np.float16 if MM_FP16 else np.float32
    shared = {
        "wih": _pack_weight_blocks(W_ih, 16, 1),
        "whh": _pack_weight_blocks(W_hh, 16, KT),
        "w1": _pack_weight_blocks(W1, KT, KT).astype(mmdt),
        "w2": _pack_weight_blocks(W2, KT, KT).astype(mmdt),
        "bg": np.ascontiguousarray((b_ih + b_hh).reshape(16, 128).T, np.float32),
        "b1": np.ascontiguousarray(b1.reshape(KT, 128).T, np.float32),
        "b2": np.ascontiguousarray(b2.reshape(KT, 128).T, np.float32),
        "b2bc": np.ascontiguousarray(
            np.repeat(b2.reshape(KT, 128).T[:, :, None], BC, axis=2)
            .reshape(128, KT * BC), np.float32),
        "dt3": np.broadcast_to(dt3, (128, 3 * NSNAP)).copy(),
    }

    in_maps = []
    for cidx in range(NCORES):
        rows = perm[cidx * BC:(cidx + 1) * BC]
        m = dict(shared)
        m["xinT"] = np.ascontiguousarray(inputs[rows].T, np.float32)
        m["hT"] = _to_hb(h[rows])
        m["cT"] = _to_hb(c[rows])
        in_maps.append(m)
    return in_maps, perm


def _post(results, perm):
    new_h = np.empty((B, H), np.float32)
    new_c = np.empty((B, H), np.float32)
    for cidx in range(NCORES):
        out = results[cidx]
        snap = out["snap"].reshape(128, KT, NSNAP)
        for j in range(BC):
            t = cidx * BC + j
            new_h[perm[t]] = snap[:, :, t].T.reshape(H)
        ncT = out["newc"].reshape(128, KT, BC)
        rows = perm[cidx * BC:(cidx + 1) * BC]
        new_c[rows] = ncT.transpose(2, 1, 0).reshape(BC, H)
    return new_h, new_c


def kernel(inputs, h, c, ts, W_ih, W_hh, b_ih, b_hh, W1, b1, W2, b2):
    in_maps, perm = _prep(inputs, h, c, ts, W_ih, W_hh, b_ih, b_hh,
                          W1, b1, W2, b2)
    if "nc" not in _CACHE:
        _CACHE["nc"] = _build_program()
    nc = _CACHE["nc"]
    res = run_bass_kernel_spmd(nc, in_maps, list(range(NCORES)), **_RUN_KWARGS)
    _LAST_RESULT.clear()
    _LAST_RESULT.append(res)
    return _post(res.results, perm)
